# revision 25
# baseline (speedup 1.0000x reference)
"""Single-head causal attention on 8 TRN2 NeuronCores.

Problem shapes (hardcoded): B=8, T=2048, C=1024, H=64, fp32 I/O.
    q = x @ Wq; k = x @ Wk; v = x @ Wv          (per batch element)
    wei = softmax(causal_mask(q @ k.T * C**-0.5))
    out = wei @ v
Sharding: pure data parallel -- one batch element per core, no collectives.

Per-core pipeline (matmuls bf16, fp32 PSUM):
  - host packs x as [128, NJ, NCB, 512] (partition, q-slice, C-chunk, t) so
    each 512-wide T-slice loads with ONE fully-contiguous DMA.
  - qkT = [Wq|Wk].T @ xT per slice (8 accumulating MMs, M=128).
  - DVE restacks from PSUM: qT2 = [qT;qT] (tensor_copy + stream_shuffle
    across partition halves), kT2 = k-block pairs stacked in halves.
  - V projection column-packed: even C-chunks -> PE cols 0-63, odd chunks
    -> cols 64-127 concurrently (tile_position), giving vA/vB partial sums;
    v natural recovered per 128-row T-block by row-packed identity MMs
    (rg0: vA chunk, rg1: vB chunk) summed pairwise on DVE into
    v1 = [v | 1] (ones column -> softmax denominators for free).
  - S^T tiles [128, 2, 512] (k-block pair in partition halves): row-packed
    K=64 MMs; ONE strided exp ACTIVATE per pair; causal masks (tri and
    [zeros|tri]) multiplied on GpSimd.
  - AV accumulates [out|denom]^T = v1.T @ P^T with lag-2 emission;
    projection work of slice j+1 is interleaved between attention pairs of
    slice j so the PE never head-of-line blocks on the exp.
  - out written TRANSPOSED + denominator ([65, T] f32); the softmax divide
    and final transpose happen on host (removes PE transposes + epilogue).
  - dummy warm-up matmuls at t=0 release the HAM clock gate early.
"""

import numpy as np
import ml_dtypes

import concourse.bass as bass
import concourse.mybir as mybir
import concourse.tile as tile
from concourse import bacc
from concourse.bass_utils import run_bass_kernel_spmd

B, T, C, H = 8, 2048, 1024, 64
NCB = C // 128          # 8 C-chunks
NT = T // 128           # 16 k-blocks of 128
NJ = T // 512           # 4 q-slices of 512
SCALE = float(C) ** -0.5  # 1/32
N_DUMMY = 32

BF16 = mybir.dt.bfloat16
F32 = mybir.dt.float32
npbf16 = ml_dtypes.bfloat16
IDENT32 = list(range(32))

# cpk layout [128, 1728] bf16 (host-packed, fully contiguous per partition):
#   [0:1024)    wqk  as (c p) h -> p (c h)   chunk c at [c*128,(c+1)*128)
#   [1024:1536) wv   as (c p) h -> p (c h)   chunk c at [1024+c*64, ...)
#   [1536:1600) i64_2
#   [1600:1728) tri
CPK_WV, CPK_I64, CPK_TRI, CPK_END = 1024, 1536, 1600, 1728


def build_attention(nc: bass.Bass, tc: tile.TileContext, ctx):
    xT_d = nc.dram_tensor("xT", [128, NJ, NCB, 512], BF16,
                          kind="ExternalInput").ap()
    cpk_d = nc.dram_tensor("cpk", [128, CPK_END], BF16,
                           kind="ExternalInput").ap()
    out_d = nc.dram_tensor("outT", [H + 1, T], F32, kind="ExternalOutput").ap()

    consts = ctx.enter_context(tc.tile_pool(name="consts", bufs=1))
    xpool = ctx.enter_context(tc.tile_pool(name="xpool", bufs=1))
    persist = ctx.enter_context(tc.tile_pool(name="persist", bufs=1))
    qpool = ctx.enter_context(tc.tile_pool(name="qpool", bufs=4))
    vpool = ctx.enter_context(tc.tile_pool(name="vpool", bufs=2))
    pts = ctx.enter_context(tc.tile_pool(name="pts", bufs=4))
    pts0 = ctx.enter_context(tc.tile_pool(name="pts0", bufs=4))
    opool = ctx.enter_context(tc.tile_pool(name="opool", bufs=2))
    ps_qk = ctx.enter_context(tc.tile_pool(name="ps_qk", bufs=1, space="PSUM"))
    ps_va = ctx.enter_context(tc.tile_pool(name="ps_va", bufs=1, space="PSUM"))
    ps_vb = ctx.enter_context(tc.tile_pool(name="ps_vb", bufs=1, space="PSUM"))
    ps_s = ctx.enter_context(tc.tile_pool(name="ps_s", bufs=2, space="PSUM"))
    ps_av = ctx.enter_context(tc.tile_pool(name="ps_av", bufs=1, space="PSUM"))

    # ---- t=0: DMAs spread across queues; consts gate the first real MM.
    # Everything host-packed contiguous (2KB/partition lines) -- rearranged
    # APs shatter into ~200B packets and take 10x longer to transfer.
    dum_sb = consts.tile([128, 128], BF16, tag="dum")
    nc.gpsimd.memset(dum_sb, 0.0)
    cpk_sb = consts.tile([128, CPK_END], BF16, tag="cpk")
    nc.scalar.dma_start(out=cpk_sb, in_=cpk_d)

    def wqk_c(c):
        return cpk_sb[:, c * 128:(c + 1) * 128]

    def wv_c(c):
        return cpk_sb[:, CPK_WV + c * H:CPK_WV + (c + 1) * H]

    i64 = cpk_sb[:, CPK_I64:CPK_TRI]  # [128, 64]: I64 stacked twice
    tri = cpk_sb[:, CPK_TRI:CPK_END]  # [128, 128] upper-tri (keep k<=q)

    # per-slice x tiles so proj(j) never waits on later slices' DMAs;
    # slice 0 in 2-chunk pieces spread across queues for fast arrival
    xs = [xpool.tile([128, NCB, 512], BF16, tag=f"x{j}", name=f"x{j}")
          for j in range(NJ)]
    # all of x on the sync ring (fastest, starts earliest), in consumption
    # order for the 0,2,3,1 attention schedule; cpk alone on scalar
    nc.sync.dma_start(out=xs[0], in_=xT_d[:, 0, :, :])
    nc.sync.dma_start(out=xs[2], in_=xT_d[:, 2, :, :])
    nc.sync.dma_start(out=xs[1], in_=xT_d[:, 1, :, :])
    nc.sync.dma_start(out=xs[3], in_=xT_d[:, 3, :, :])

    kT2 = persist.tile([128, NT // 2, 128], BF16, tag="kT2")
    v1 = persist.tile([128, NT, H + 1], BF16, tag="v1")
    nc.vector.memset(v1, 1.0)

    # ---- HAM warm-up: keep the PE busy while the first DMAs land ----
    dum_ps = ps_va.tile([128, 128], F32, tag="va", name="dum_ps")
    for i in range(N_DUMMY):
        nc.tensor.matmul(dum_ps, lhsT=dum_sb, rhs=dum_sb,
                         start=True, stop=True)

    # ------------------------------------------------------------------
    def emit_prologue(j):
        """Build slice j's projection emission closures (~1-2 PE matmuls
        each, so they can fill exp-wait gaps inside the previous slice's
        attention). Returns (items, qT2)."""
        qk_items = []
        v_items = []
        qk = ps_qk.tile([128, 512], F32, tag="qk", name=f"qk{j}")
        for c in range(NCB):
            qk_items.append(lambda c=c, qk=qk: nc.tensor.matmul(
                qk, lhsT=wqk_c(c), rhs=xs[j][:, c, :],
                start=(c == 0), stop=(c == NCB - 1)))

        # qkst: one big PSUM->bf16 cast; rows 0-63 double as the S-matmul's
        # lo-half rhs (q), rows 64-127 stage k for the kT2 restacks.
        # (stream_shuffle cannot cast -- s4d4_tr_same_src_dst_type.)
        qkst = qpool.tile([128, 512], BF16, tag="qkst", name=f"qkst{j}")
        qT2 = qpool.tile([128, 512], BF16, tag="qT2", name=f"qT2_{j}")

        def restack(qk=qk, qkst=qkst, qT2=qT2, j=j):
            nc.vector.tensor_copy(qkst, qk)
            nc.vector.stream_shuffle(qT2[64:128, :], qkst[0:64, :], IDENT32)
            for b in range(4):          # k-block 4j+b -> pair m, half b%2
                m = (4 * j + b) // 2
                src = qkst[64:128, b * 128:(b + 1) * 128]
                if b % 2 == 0:
                    nc.vector.stream_shuffle(kT2[0:64, m, :], src, IDENT32)
                else:
                    nc.vector.tensor_copy(kT2[64:128, m, :], src)
        qk_items.append(restack)
        qts = (qkst, qT2)

        # even C-chunks -> PE col group 0-1 -> bank A; odd chunks -> col
        # group 2-3 -> bank B; the two chains run concurrently (separate
        # banks so each keeps its own psum accumulation group)
        vpsa = ps_va.tile([128, 512], F32, tag="va", name=f"vpsa{j}")
        vpsb = ps_vb.tile([128, 512], F32, tag="vb", name=f"vpsb{j}")
        for r in range(4):
            def vproj(r=r, vpsa=vpsa, vpsb=vpsb, j=j):
                nc.tensor.matmul(vpsa[0:64, :], lhsT=wv_c(2 * r),
                                 rhs=xs[j][:, 2 * r, :],
                                 start=(r == 0), stop=(r == 3),
                                 tile_position=(0, 0))
                nc.tensor.matmul(vpsb[64:128, :], lhsT=wv_c(2 * r + 1),
                                 rhs=xs[j][:, 2 * r + 1, :],
                                 start=(r == 0), stop=(r == 3),
                                 tile_position=(0, 64))
            v_items.append(vproj)

        vth = vpool.tile([128, 512], BF16, tag="vth", name=f"vth{j}")

        def vcast(vpsa=vpsa, vpsb=vpsb, vth=vth):
            nc.vector.tensor_copy(vth[0:64, :], vpsa[0:64, :])
            nc.vector.tensor_copy(vth[64:128, :], vpsb[64:128, :])
        v_items.append(vcast)

        for t in range(4):
            def vnat(t=t, vth=vth, j=j):
                vpa = ps_va.tile([128, H], F32, tag="va", name=f"vpa{j}_{t}")
                vpb = ps_vb.tile([128, H], F32, tag="vb", name=f"vpb{j}_{t}")
                nc.tensor.matmul(vpa,
                                 lhsT=vth[0:64, t * 128:(t + 1) * 128],
                                 rhs=i64[0:64, :], start=True, stop=True)
                nc.tensor.matmul(vpb,
                                 lhsT=vth[64:128, t * 128:(t + 1) * 128],
                                 rhs=i64[64:128, :], start=True, stop=True)
                # DVE may read only ONE PSUM operand per instruction
                nc.vector.tensor_copy(v1[:, 4 * j + t, 0:H], vpa)
                nc.vector.tensor_add(v1[:, 4 * j + t, 0:H],
                                     v1[:, 4 * j + t, 0:H], vpb)
            v_items.append(vnat)
        return qk_items, v_items, qts

    # ------------------------------------------------------------------
    def emit_s_pair(j, m, qts, pt_pool, done):
        """S^T pair m of slice j: 2 row-packed MMs + one exp + masks.
        Appends (pt, n0, m) to `done` for the later AV phase."""
        qkst, qT2 = qts
        n0 = (2 * m - 4 * j) * 128 if m >= 2 * j else 0
        sp = ps_s.tile([128, 2, 512], F32, tag="s", name=f"sp{j}_{m}")
        pt = pt_pool.tile([128, 2, 512], BF16, tag="pt", name=f"pt{j}_{m}")
        nc.tensor.matmul(sp[:, 0, n0:512], lhsT=kT2[0:64, m, :],
                         rhs=qkst[0:64, n0:512],
                         start=True, stop=True)
        nc.tensor.matmul(sp[:, 1, n0:512], lhsT=kT2[64:128, m, :],
                         rhs=qT2[64:128, n0:512],
                         start=True, stop=True)
        nc.scalar.activation(pt[:, :, n0:512], sp[:, :, n0:512],
                             mybir.ActivationFunctionType.Exp,
                             scale=SCALE)
        if m >= 2 * j:  # diagonal pair: tri-mask each diagonal block
            # (the exp'd strip [n0:n0+128) of the hi half is never read
            # by AV -- av starts at n0+128 there -- so no zeroing needed)
            nc.vector.tensor_mul(pt[:, 0, n0:n0 + 128],
                                 pt[:, 0, n0:n0 + 128], tri)
            nc.vector.tensor_mul(pt[:, 1, n0 + 128:n0 + 256],
                                 pt[:, 1, n0 + 128:n0 + 256], tri)
        done.append((pt, n0, m))

    def make_av_emitter(j):
        av = ps_av.tile([H + 1, 512], F32, tag="av", name=f"av{j}")

        def emit_av(pt, n0, m):
            i_lo, i_hi = 2 * m, 2 * m + 1
            nc.tensor.matmul(av[:, n0:512], lhsT=v1[:, i_lo, :],
                             rhs=pt[:, 0, n0:512],
                             start=(i_lo == 0), stop=False)
            n0h = n0 + 128 if m >= 2 * j else 0
            nc.tensor.matmul(av[:, n0h:512], lhsT=v1[:, i_hi, :],
                             rhs=pt[:, 1, n0h:512],
                             start=False, stop=(i_hi == 4 * j + 3))
        return av, emit_av

    def emit_epilogue(j, av):
        outsb = opool.tile([H + 1, 512], F32, tag="osb", name=f"osb{j}")
        nc.vector.tensor_copy(outsb, av)
        nc.sync.dma_start(out=out_d[:, j * 512:(j + 1) * 512], in_=outsb)

    def emit_attention(j, qts, pending):
        """S^T pairs + exp + masks + lag-2 AV, with `pending` (other work:
        next slice's projections, or the tail slice's S pairs) interleaved
        between pairs to keep the PE fed during exp waits."""
        av, emit_av = make_av_emitter(j)
        npair = 2 * j + 2
        done = []
        navd = 0
        for m in range(npair):
            emit_s_pair(j, m, qts, pts, done)
            if pending:
                take = max(1, -(-len(pending) // (npair - m)))
                for _ in range(min(take, len(pending))):
                    pending.pop(0)()
            if len(done) - navd > 2:
                emit_av(*done[navd])
                navd += 1
        for a in done[navd:]:
            emit_av(*a)
        for fn in pending:
            fn()
        emit_epilogue(j, av)

    # ------------------------------------------------------------------
    # Attention slice order 0, 2, 3, 1: att(0) needs only x-slice 0, so the
    # exp stream starts as early as the DMAs allow, and the tail slice is a
    # SHORT one (4 exp pairs); slice 1's S/exp work rides inside slice 3's
    # otherwise exp-bound stretch. V work and later q/k projections ride
    # the attention interleave so the PE never head-of-line blocks.
    qk0, v0, qts0 = emit_prologue(0)
    for fn in qk0:
        fn()
    qk1, v1i, qts1 = emit_prologue(1)
    qk2, v2, qts2 = emit_prologue(2)
    emit_attention(0, qts0, v0 + qk2 + qk1)
    qk3, v3, qts3 = emit_prologue(3)
    emit_attention(2, qts2, v1i + v2 + qk3)
    done1 = []
    s1_items = [lambda m=m: emit_s_pair(1, m, qts1, pts0, done1)
                for m in range(4)]
    emit_attention(3, qts3, v3 + s1_items)
    av1, emit_av1 = make_av_emitter(1)
    for a in done1:
        emit_av1(*a)
    emit_epilogue(1, av1)


_CACHED = {}


def _get_nc():
    if "nc" not in _CACHED:
        from contextlib import ExitStack
        nc = bacc.Bacc("TRN2", target_bir_lowering=False, debug=False,
                       num_devices=B)
        with tile.TileContext(nc) as tc:
            with ExitStack() as ctx:
                build_attention(nc, tc, ctx)
        nc.compile()
        _CACHED["nc"] = nc
    return _CACHED["nc"]


def make_cpk(Wq, Wk, Wv):
    wqk = np.concatenate([np.asarray(Wq), np.asarray(Wk)],
                         axis=1).astype(np.float32)        # [C, 128]
    wqk_p = wqk.reshape(NCB, 128, 128).transpose(1, 0, 2).reshape(128, 1024)
    wv_p = np.asarray(Wv, dtype=np.float32).reshape(
        NCB, 128, H).transpose(1, 0, 2).reshape(128, NCB * H)
    i64_2 = np.concatenate([np.eye(64, dtype=np.float32)] * 2, axis=0)
    tri = np.triu(np.ones((128, 128), dtype=np.float32))
    cpk = np.concatenate([wqk_p, wv_p, i64_2, tri], axis=1)
    assert cpk.shape == (128, CPK_END)
    return np.ascontiguousarray(cpk).astype(npbf16)


def prep_in_maps(inputs, Wq, Wk, Wv):
    cpk = make_cpk(Wq, Wk, Wv)
    in_maps = []
    for b in range(B):
        xb = np.asarray(inputs[b], dtype=np.float32).astype(npbf16)
        # [T, C] -> [C, T] -> (c p) (j t) -> [p j c t]
        xh = np.ascontiguousarray(
            xb.T.reshape(NCB, 128, NJ, 512).transpose(1, 2, 0, 3))
        in_maps.append({"xT": xh, "cpk": cpk})
    return in_maps


def finish(res):
    outs = []
    for b in range(B):
        oT = np.asarray(res.results[b]["outT"], dtype=np.float32)
        outs.append((oT[:H] / oT[H:H + 1]).T)
    return np.stack(outs, axis=0).astype(np.float32)


def kernel(inputs, Wq, Wk, Wv):
    in_maps = prep_in_maps(np.asarray(inputs), Wq, Wk, Wv)
    nc = _get_nc()
    res = run_bass_kernel_spmd(nc, in_maps, core_ids=list(range(B)))
    return finish(res)


# revision 26
# speedup vs baseline: 1.0084x; 1.0084x over previous
"""Single-head causal attention on 8 TRN2 NeuronCores.

Problem shapes (hardcoded): B=8, T=2048, C=1024, H=64, fp32 I/O.
    q = x @ Wq; k = x @ Wk; v = x @ Wv          (per batch element)
    wei = softmax(causal_mask(q @ k.T * C**-0.5))
    out = wei @ v
Sharding: pure data parallel -- one batch element per core, no collectives.

Per-core pipeline (matmuls bf16, fp32 PSUM):
  - host packs x as [128, NJ, NCB, 512] (partition, q-slice, C-chunk, t) so
    each 512-wide T-slice loads with ONE fully-contiguous DMA.
  - qkT = [Wq|Wk].T @ xT per slice (8 accumulating MMs, M=128).
  - DVE restacks from PSUM: qT2 = [qT;qT] (tensor_copy + stream_shuffle
    across partition halves), kT2 = k-block pairs stacked in halves.
  - V projection column-packed: even C-chunks -> PE cols 0-63, odd chunks
    -> cols 64-127 concurrently (tile_position), giving vA/vB partial sums;
    v natural recovered per 128-row T-block by row-packed identity MMs
    (rg0: vA chunk, rg1: vB chunk) summed pairwise on DVE into
    v1 = [v | 1] (ones column -> softmax denominators for free).
  - S^T tiles [128, 2, 512] (k-block pair in partition halves): row-packed
    K=64 MMs; ONE strided exp ACTIVATE per pair; causal masks (tri and
    [zeros|tri]) multiplied on GpSimd.
  - AV accumulates [out|denom]^T = v1.T @ P^T with lag-2 emission;
    projection work of slice j+1 is interleaved between attention pairs of
    slice j so the PE never head-of-line blocks on the exp.
  - out written TRANSPOSED + denominator ([65, T] f32); the softmax divide
    and final transpose happen on host (removes PE transposes + epilogue).
  - dummy warm-up matmuls at t=0 release the HAM clock gate early.
"""

import numpy as np
import ml_dtypes

import concourse.bass as bass
import concourse.mybir as mybir
import concourse.tile as tile
from concourse import bacc
from concourse.bass_utils import run_bass_kernel_spmd

B, T, C, H = 8, 2048, 1024, 64
NCB = C // 128          # 8 C-chunks
NT = T // 128           # 16 k-blocks of 128
NJ = T // 512           # 4 q-slices of 512
SCALE = float(C) ** -0.5  # 1/32
N_DUMMY = 30

BF16 = mybir.dt.bfloat16
F32 = mybir.dt.float32
npbf16 = ml_dtypes.bfloat16
IDENT32 = list(range(32))

# cpk layout [128, 1728] bf16 (host-packed, fully contiguous per partition):
#   [0:1024)    wqk  as (c p) h -> p (c h)   chunk c at [c*128,(c+1)*128)
#   [1024:1536) wv   as (c p) h -> p (c h)   chunk c at [1024+c*64, ...)
#   [1536:1600) i64_2
#   [1600:1728) tri
CPK_WV, CPK_I64, CPK_TRI, CPK_END = 1024, 1536, 1600, 1728


def build_attention(nc: bass.Bass, tc: tile.TileContext, ctx):
    xT_d = nc.dram_tensor("xT", [128, NJ, NCB, 512], BF16,
                          kind="ExternalInput").ap()
    cpk_d = nc.dram_tensor("cpk", [128, CPK_END], BF16,
                           kind="ExternalInput").ap()
    out_d = nc.dram_tensor("outT", [H + 1, T], F32, kind="ExternalOutput").ap()

    consts = ctx.enter_context(tc.tile_pool(name="consts", bufs=1))
    xpool = ctx.enter_context(tc.tile_pool(name="xpool", bufs=1))
    persist = ctx.enter_context(tc.tile_pool(name="persist", bufs=1))
    qpool = ctx.enter_context(tc.tile_pool(name="qpool", bufs=4))
    vpool = ctx.enter_context(tc.tile_pool(name="vpool", bufs=2))
    pts = ctx.enter_context(tc.tile_pool(name="pts", bufs=4))
    pts0 = ctx.enter_context(tc.tile_pool(name="pts0", bufs=4))
    opool = ctx.enter_context(tc.tile_pool(name="opool", bufs=2))
    ps_qk = ctx.enter_context(tc.tile_pool(name="ps_qk", bufs=1, space="PSUM"))
    ps_va = ctx.enter_context(tc.tile_pool(name="ps_va", bufs=1, space="PSUM"))
    ps_vb = ctx.enter_context(tc.tile_pool(name="ps_vb", bufs=1, space="PSUM"))
    ps_s = ctx.enter_context(tc.tile_pool(name="ps_s", bufs=2, space="PSUM"))
    ps_av = ctx.enter_context(tc.tile_pool(name="ps_av", bufs=1, space="PSUM"))

    # ---- t=0: DMAs spread across queues; consts gate the first real MM.
    # Everything host-packed contiguous (2KB/partition lines) -- rearranged
    # APs shatter into ~200B packets and take 10x longer to transfer.
    dum_sb = consts.tile([128, 128], BF16, tag="dum")
    nc.gpsimd.memset(dum_sb, 0.0)
    cpk_sb = consts.tile([128, CPK_END], BF16, tag="cpk")
    # wqk half rides FIRST on the fast sync ring (gates the first matmul);
    # the rest (wv/i64/tri, needed later) goes on the scalar ring
    nc.sync.dma_start(out=cpk_sb[:, 0:CPK_WV], in_=cpk_d[:, 0:CPK_WV])
    nc.scalar.dma_start(out=cpk_sb[:, CPK_WV:], in_=cpk_d[:, CPK_WV:])

    def wqk_c(c):
        return cpk_sb[:, c * 128:(c + 1) * 128]

    def wv_c(c):
        return cpk_sb[:, CPK_WV + c * H:CPK_WV + (c + 1) * H]

    i64 = cpk_sb[:, CPK_I64:CPK_TRI]  # [128, 64]: I64 stacked twice
    tri = cpk_sb[:, CPK_TRI:CPK_END]  # [128, 128] upper-tri (keep k<=q)

    # per-slice x tiles so proj(j) never waits on later slices' DMAs;
    # slice 0 in 2-chunk pieces spread across queues for fast arrival
    xs = [xpool.tile([128, NCB, 512], BF16, tag=f"x{j}", name=f"x{j}")
          for j in range(NJ)]
    # all of x on the sync ring (fastest, starts earliest) in consumption
    # order; slice 0 split in halves so the first proj MMs start sooner
    nc.sync.dma_start(out=xs[0][:, 0:4, :], in_=xT_d[:, 0, 0:4, :])
    nc.sync.dma_start(out=xs[0][:, 4:8, :], in_=xT_d[:, 0, 4:8, :])
    for j in range(1, NJ):
        nc.sync.dma_start(out=xs[j], in_=xT_d[:, j, :, :])

    kT2 = persist.tile([128, NT // 2, 128], BF16, tag="kT2")
    v1 = persist.tile([128, NT, H + 1], BF16, tag="v1")
    nc.vector.memset(v1, 1.0)

    # ---- HAM warm-up: keep the PE busy while the first DMAs land ----
    dum_ps = ps_va.tile([128, 128], F32, tag="va", name="dum_ps")
    for i in range(N_DUMMY):
        nc.tensor.matmul(dum_ps, lhsT=dum_sb, rhs=dum_sb,
                         start=True, stop=True)

    # ------------------------------------------------------------------
    def emit_prologue(j):
        """Build slice j's projection emission closures (~1-2 PE matmuls
        each, so they can fill exp-wait gaps inside the previous slice's
        attention). Returns (items, qT2)."""
        qk_items = []
        v_items = []
        qk = ps_qk.tile([128, 512], F32, tag="qk", name=f"qk{j}")
        for c in range(NCB):
            qk_items.append(lambda c=c, qk=qk: nc.tensor.matmul(
                qk, lhsT=wqk_c(c), rhs=xs[j][:, c, :],
                start=(c == 0), stop=(c == NCB - 1)))

        # qkst: one big PSUM->bf16 cast; rows 0-63 double as the S-matmul's
        # lo-half rhs (q), rows 64-127 stage k for the kT2 restacks.
        # (stream_shuffle cannot cast -- s4d4_tr_same_src_dst_type.)
        qkst = qpool.tile([128, 512], BF16, tag="qkst", name=f"qkst{j}")
        qT2 = qpool.tile([128, 512], BF16, tag="qT2", name=f"qT2_{j}")

        def restack(qk=qk, qkst=qkst, qT2=qT2, j=j):
            nc.vector.tensor_copy(qkst, qk)
            nc.vector.stream_shuffle(qT2[64:128, :], qkst[0:64, :], IDENT32)
            for b in range(4):          # k-block 4j+b -> pair m, half b%2
                m = (4 * j + b) // 2
                src = qkst[64:128, b * 128:(b + 1) * 128]
                if b % 2 == 0:
                    nc.vector.stream_shuffle(kT2[0:64, m, :], src, IDENT32)
                else:
                    nc.vector.tensor_copy(kT2[64:128, m, :], src)
        qk_items.append(restack)
        qts = (qkst, qT2)

        # even C-chunks -> PE col group 0-1 -> bank A; odd chunks -> col
        # group 2-3 -> bank B; the two chains run concurrently (separate
        # banks so each keeps its own psum accumulation group)
        vpsa = ps_va.tile([128, 512], F32, tag="va", name=f"vpsa{j}")
        vpsb = ps_vb.tile([128, 512], F32, tag="vb", name=f"vpsb{j}")
        for r in range(4):
            def vproj(r=r, vpsa=vpsa, vpsb=vpsb, j=j):
                nc.tensor.matmul(vpsa[0:64, :], lhsT=wv_c(2 * r),
                                 rhs=xs[j][:, 2 * r, :],
                                 start=(r == 0), stop=(r == 3),
                                 tile_position=(0, 0))
                nc.tensor.matmul(vpsb[64:128, :], lhsT=wv_c(2 * r + 1),
                                 rhs=xs[j][:, 2 * r + 1, :],
                                 start=(r == 0), stop=(r == 3),
                                 tile_position=(0, 64))
            v_items.append(vproj)

        vth = vpool.tile([128, 512], BF16, tag="vth", name=f"vth{j}")

        def vcast(vpsa=vpsa, vpsb=vpsb, vth=vth):
            nc.vector.tensor_copy(vth[0:64, :], vpsa[0:64, :])
            nc.vector.tensor_copy(vth[64:128, :], vpsb[64:128, :])
        v_items.append(vcast)

        for t in range(4):
            def vnat(t=t, vth=vth, j=j):
                vpa = ps_va.tile([128, H], F32, tag="va", name=f"vpa{j}_{t}")
                vpb = ps_vb.tile([128, H], F32, tag="vb", name=f"vpb{j}_{t}")
                nc.tensor.matmul(vpa,
                                 lhsT=vth[0:64, t * 128:(t + 1) * 128],
                                 rhs=i64[0:64, :], start=True, stop=True)
                nc.tensor.matmul(vpb,
                                 lhsT=vth[64:128, t * 128:(t + 1) * 128],
                                 rhs=i64[64:128, :], start=True, stop=True)
                # DVE may read only ONE PSUM operand per instruction
                nc.vector.tensor_copy(v1[:, 4 * j + t, 0:H], vpa)
                nc.vector.tensor_add(v1[:, 4 * j + t, 0:H],
                                     v1[:, 4 * j + t, 0:H], vpb)
            v_items.append(vnat)
        return qk_items, v_items, qts

    # ------------------------------------------------------------------
    def emit_s_pair(j, m, qts, pt_pool, done):
        """S^T pair m of slice j: 2 row-packed MMs + one exp + masks.
        Appends (pt, n0, m) to `done` for the later AV phase."""
        qkst, qT2 = qts
        n0 = (2 * m - 4 * j) * 128 if m >= 2 * j else 0
        sp = ps_s.tile([128, 2, 512], F32, tag="s", name=f"sp{j}_{m}")
        pt = pt_pool.tile([128, 2, 512], BF16, tag="pt", name=f"pt{j}_{m}")
        nc.tensor.matmul(sp[:, 0, n0:512], lhsT=kT2[0:64, m, :],
                         rhs=qkst[0:64, n0:512],
                         start=True, stop=True)
        nc.tensor.matmul(sp[:, 1, n0:512], lhsT=kT2[64:128, m, :],
                         rhs=qT2[64:128, n0:512],
                         start=True, stop=True)
        nc.scalar.activation(pt[:, :, n0:512], sp[:, :, n0:512],
                             mybir.ActivationFunctionType.Exp,
                             scale=SCALE)
        if m >= 2 * j:  # diagonal pair: tri-mask each diagonal block
            # (the exp'd strip [n0:n0+128) of the hi half is never read
            # by AV -- av starts at n0+128 there -- so no zeroing needed)
            nc.vector.tensor_mul(pt[:, 0, n0:n0 + 128],
                                 pt[:, 0, n0:n0 + 128], tri)
            nc.vector.tensor_mul(pt[:, 1, n0 + 128:n0 + 256],
                                 pt[:, 1, n0 + 128:n0 + 256], tri)
        done.append((pt, n0, m))

    def make_av_emitter(j):
        av = ps_av.tile([H + 1, 512], F32, tag="av", name=f"av{j}")

        def emit_av(pt, n0, m):
            i_lo, i_hi = 2 * m, 2 * m + 1
            nc.tensor.matmul(av[:, n0:512], lhsT=v1[:, i_lo, :],
                             rhs=pt[:, 0, n0:512],
                             start=(i_lo == 0), stop=False)
            n0h = n0 + 128 if m >= 2 * j else 0
            nc.tensor.matmul(av[:, n0h:512], lhsT=v1[:, i_hi, :],
                             rhs=pt[:, 1, n0h:512],
                             start=False, stop=(i_hi == 4 * j + 3))
        return av, emit_av

    def emit_epilogue(j, av):
        outsb = opool.tile([H + 1, 512], F32, tag="osb", name=f"osb{j}")
        nc.vector.tensor_copy(outsb, av)
        nc.sync.dma_start(out=out_d[:, j * 512:(j + 1) * 512], in_=outsb)

    def emit_attention(j, qts, pending):
        """S^T pairs + exp + masks + lag-2 AV, with `pending` (other work:
        next slice's projections, or the tail slice's S pairs) interleaved
        between pairs to keep the PE fed during exp waits."""
        av, emit_av = make_av_emitter(j)
        npair = 2 * j + 2
        done = []
        navd = 0
        for m in range(npair):
            emit_s_pair(j, m, qts, pts, done)
            if pending:
                take = max(1, -(-len(pending) // (npair - m)))
                for _ in range(min(take, len(pending))):
                    pending.pop(0)()
            if len(done) - navd > 2:
                emit_av(*done[navd])
                navd += 1
        for a in done[navd:]:
            emit_av(*a)
        for fn in pending:
            fn()
        emit_epilogue(j, av)

    # ------------------------------------------------------------------
    # Attention runs in natural slice order 0,1,2,3 -- this matches the
    # sequential arrival of the x-slice DMAs, so the exp stream on ScalarE
    # (the serial 21us backbone of the kernel) never starves. V work and
    # the next slice's q/k projection ride the attention interleave.
    qk0, v0, qts0 = emit_prologue(0)
    for fn in qk0:
        fn()
    qk1, v1i, qts1 = emit_prologue(1)
    emit_attention(0, qts0, v0 + qk1)
    qk2, v2, qts2 = emit_prologue(2)
    emit_attention(1, qts1, v1i + qk2)
    qk3, v3, qts3 = emit_prologue(3)
    emit_attention(2, qts2, v2 + qk3)
    emit_attention(3, qts3, v3)


_CACHED = {}


def _get_nc():
    if "nc" not in _CACHED:
        from contextlib import ExitStack
        nc = bacc.Bacc("TRN2", target_bir_lowering=False, debug=False,
                       num_devices=B)
        with tile.TileContext(nc) as tc:
            with ExitStack() as ctx:
                build_attention(nc, tc, ctx)
        nc.compile()
        _CACHED["nc"] = nc
    return _CACHED["nc"]


def make_cpk(Wq, Wk, Wv):
    wqk = np.concatenate([np.asarray(Wq), np.asarray(Wk)],
                         axis=1).astype(np.float32)        # [C, 128]
    wqk_p = wqk.reshape(NCB, 128, 128).transpose(1, 0, 2).reshape(128, 1024)
    wv_p = np.asarray(Wv, dtype=np.float32).reshape(
        NCB, 128, H).transpose(1, 0, 2).reshape(128, NCB * H)
    i64_2 = np.concatenate([np.eye(64, dtype=np.float32)] * 2, axis=0)
    tri = np.triu(np.ones((128, 128), dtype=np.float32))
    cpk = np.concatenate([wqk_p, wv_p, i64_2, tri], axis=1)
    assert cpk.shape == (128, CPK_END)
    return np.ascontiguousarray(cpk).astype(npbf16)


def prep_in_maps(inputs, Wq, Wk, Wv):
    cpk = make_cpk(Wq, Wk, Wv)
    in_maps = []
    for b in range(B):
        xb = np.asarray(inputs[b], dtype=np.float32).astype(npbf16)
        # [T, C] -> [C, T] -> (c p) (j t) -> [p j c t]
        xh = np.ascontiguousarray(
            xb.T.reshape(NCB, 128, NJ, 512).transpose(1, 2, 0, 3))
        in_maps.append({"xT": xh, "cpk": cpk})
    return in_maps


def finish(res):
    outs = []
    for b in range(B):
        oT = np.asarray(res.results[b]["outT"], dtype=np.float32)
        outs.append((oT[:H] / oT[H:H + 1]).T)
    return np.stack(outs, axis=0).astype(np.float32)


def kernel(inputs, Wq, Wk, Wv):
    in_maps = prep_in_maps(np.asarray(inputs), Wq, Wk, Wv)
    nc = _get_nc()
    res = run_bass_kernel_spmd(nc, in_maps, core_ids=list(range(B)))
    return finish(res)


# revision 27
# speedup vs baseline: 1.1269x; 1.1175x over previous
"""Single-head causal attention on 8 TRN2 NeuronCores.

Problem shapes (hardcoded): B=8, T=2048, C=1024, H=64, fp32 I/O.
    q = x @ Wq; k = x @ Wk; v = x @ Wv          (per batch element)
    wei = softmax(causal_mask(q @ k.T * C**-0.5))
    out = wei @ v
Sharding: pure data parallel -- one batch element per core, no collectives.

Per-core pipeline (matmuls bf16, fp32 PSUM):
  - host packs x as [128, NJ, NCB, 512] (partition, q-slice, C-chunk, t) so
    each 512-wide T-slice loads with ONE fully-contiguous DMA.
  - qkT = [Wq|Wk].T @ xT per slice (8 accumulating MMs, M=128).
  - DVE restacks from PSUM: qT2 = [qT;qT] (tensor_copy + stream_shuffle
    across partition halves), kT2 = k-block pairs stacked in halves.
  - V projection column-packed: even C-chunks -> PE cols 0-63, odd chunks
    -> cols 64-127 concurrently (tile_position), giving vA/vB partial sums;
    v natural recovered per 128-row T-block by row-packed identity MMs
    (rg0: vA chunk, rg1: vB chunk) summed pairwise on DVE into
    v1 = [v | 1] (ones column -> softmax denominators for free).
  - S^T tiles [128, 2, 512] (k-block pair in partition halves): row-packed
    K=64 MMs; ONE strided exp ACTIVATE per pair; causal masks (tri and
    [zeros|tri]) multiplied on GpSimd.
  - AV accumulates [out|denom]^T = v1.T @ P^T with lag-2 emission;
    projection work of slice j+1 is interleaved between attention pairs of
    slice j so the PE never head-of-line blocks on the exp.
  - out written TRANSPOSED + denominator ([65, T] f32); the softmax divide
    and final transpose happen on host (removes PE transposes + epilogue).
  - dummy warm-up matmuls at t=0 release the HAM clock gate early.
"""

import numpy as np
import ml_dtypes

import concourse.bass as bass
import concourse.mybir as mybir
import concourse.tile as tile
from concourse import bacc
from concourse.bass_utils import run_bass_kernel_spmd

B, T, C, H = 8, 2048, 1024, 64
NCB = C // 128          # 8 C-chunks
NT = T // 128           # 16 k-blocks of 128
NJ = T // 512           # 4 q-slices of 512
SCALE = float(C) ** -0.5  # 1/32
N_DUMMY = 30

BF16 = mybir.dt.bfloat16
F32 = mybir.dt.float32
npbf16 = ml_dtypes.bfloat16
IDENT32 = list(range(32))

# cpk layout [128, 1728] bf16 (host-packed, fully contiguous per partition):
#   [0:1024)    wqk  as (c p) h -> p (c h)   chunk c at [c*128,(c+1)*128)
#   [1024:1536) wv   as (c p) h -> p (c h)   chunk c at [1024+c*64, ...)
#   [1536:1600) i64_2
#   [1600:1728) tri
CPK_WV, CPK_I64, CPK_TRI, CPK_END = 1024, 1536, 1600, 1728


def build_attention(nc: bass.Bass, tc: tile.TileContext, ctx):
    xT_d = nc.dram_tensor("xT", [128, NJ, NCB, 512], BF16,
                          kind="ExternalInput").ap()
    cpk_d = nc.dram_tensor("cpk", [128, CPK_END], BF16,
                           kind="ExternalInput").ap()
    out_d = nc.dram_tensor("outT", [H + 1, T], F32, kind="ExternalOutput").ap()

    consts = ctx.enter_context(tc.tile_pool(name="consts", bufs=1))
    xpool = ctx.enter_context(tc.tile_pool(name="xpool", bufs=1))
    persist = ctx.enter_context(tc.tile_pool(name="persist", bufs=1))
    qpool = ctx.enter_context(tc.tile_pool(name="qpool", bufs=4))
    vpool = ctx.enter_context(tc.tile_pool(name="vpool", bufs=2))
    pts = ctx.enter_context(tc.tile_pool(name="pts", bufs=4))
    pts0 = ctx.enter_context(tc.tile_pool(name="pts0", bufs=4))
    opool = ctx.enter_context(tc.tile_pool(name="opool", bufs=2))
    ps_qk = ctx.enter_context(tc.tile_pool(name="ps_qk", bufs=1, space="PSUM"))
    ps_va = ctx.enter_context(tc.tile_pool(name="ps_va", bufs=1, space="PSUM"))
    ps_vb = ctx.enter_context(tc.tile_pool(name="ps_vb", bufs=1, space="PSUM"))
    ps_s = ctx.enter_context(tc.tile_pool(name="ps_s", bufs=2, space="PSUM"))
    ps_av = ctx.enter_context(tc.tile_pool(name="ps_av", bufs=1, space="PSUM"))

    # ---- t=0: DMAs spread across queues; consts gate the first real MM.
    # Everything host-packed contiguous (2KB/partition lines) -- rearranged
    # APs shatter into ~200B packets and take 10x longer to transfer.
    dum_sb = consts.tile([128, 128], BF16, tag="dum")
    nc.gpsimd.memset(dum_sb, 0.0)
    cpk_sb = consts.tile([128, CPK_END], BF16, tag="cpk")
    # wqk half rides FIRST on the fast sync ring (gates the first matmul);
    # the rest (wv/i64/tri, needed later) goes on the scalar ring
    nc.sync.dma_start(out=cpk_sb[:, 0:CPK_WV], in_=cpk_d[:, 0:CPK_WV])
    nc.scalar.dma_start(out=cpk_sb[:, CPK_WV:], in_=cpk_d[:, CPK_WV:])

    def wqk_c(c):
        return cpk_sb[:, c * 128:(c + 1) * 128]

    def wv_c(c):
        return cpk_sb[:, CPK_WV + c * H:CPK_WV + (c + 1) * H]

    i64 = cpk_sb[:, CPK_I64:CPK_TRI]  # [128, 64]: I64 stacked twice
    tri = cpk_sb[:, CPK_TRI:CPK_END]  # [128, 128] upper-tri (keep k<=q)

    # per-slice x tiles so proj(j) never waits on later slices' DMAs;
    # slice 0 in 2-chunk pieces spread across queues for fast arrival
    xs = [xpool.tile([128, NCB, 512], BF16, tag=f"x{j}", name=f"x{j}")
          for j in range(NJ)]
    # all of x on the sync ring (fastest, starts earliest) in consumption
    # order; slice 0 split in halves so the first proj MMs start sooner
    nc.sync.dma_start(out=xs[0][:, 0:4, :], in_=xT_d[:, 0, 0:4, :])
    nc.sync.dma_start(out=xs[0][:, 4:8, :], in_=xT_d[:, 0, 4:8, :])
    for j in range(1, NJ):
        nc.sync.dma_start(out=xs[j], in_=xT_d[:, j, :, :])

    kT2 = persist.tile([128, NT // 2, 128], BF16, tag="kT2")
    v1 = persist.tile([128, NT, H + 1], BF16, tag="v1")
    nc.vector.memset(v1, 1.0)

    # ---- HAM warm-up: keep the PE busy while the first DMAs land ----
    dum_ps = ps_va.tile([128, 128], F32, tag="va", name="dum_ps")
    for i in range(N_DUMMY):
        nc.tensor.matmul(dum_ps, lhsT=dum_sb, rhs=dum_sb,
                         start=True, stop=True)

    # ------------------------------------------------------------------
    def emit_prologue(j):
        """Build slice j's projection emission closures (~1-2 PE matmuls
        each, so they can fill exp-wait gaps inside the previous slice's
        attention). Returns (items, qT2)."""
        qk_items = []     # q/k proj MMs + restack (x-gated only)
        vp_items = []     # V proj MMs (x-gated only)
        v_items = []      # vcast + vnat (DVE-dependent -- emit LAST)
        qk = ps_qk.tile([128, 512], F32, tag="qk", name=f"qk{j}")
        for c in range(NCB):
            qk_items.append(lambda c=c, qk=qk: nc.tensor.matmul(
                qk, lhsT=wqk_c(c), rhs=xs[j][:, c, :],
                start=(c == 0), stop=(c == NCB - 1)))

        # qkst: one big PSUM->bf16 cast; rows 0-63 double as the S-matmul's
        # lo-half rhs (q), rows 64-127 stage k for the kT2 restacks.
        # (stream_shuffle cannot cast -- s4d4_tr_same_src_dst_type.)
        qkst = qpool.tile([128, 512], BF16, tag="qkst", name=f"qkst{j}")
        qT2 = qpool.tile([128, 512], BF16, tag="qT2", name=f"qT2_{j}")

        def restack(qk=qk, qkst=qkst, qT2=qT2, j=j):
            nc.vector.tensor_copy(qkst, qk)
            nc.vector.stream_shuffle(qT2[64:128, :], qkst[0:64, :], IDENT32)
            for b in range(4):          # k-block 4j+b -> pair m, half b%2
                m = (4 * j + b) // 2
                src = qkst[64:128, b * 128:(b + 1) * 128]
                if b % 2 == 0:
                    nc.vector.stream_shuffle(kT2[0:64, m, :], src, IDENT32)
                else:
                    nc.vector.tensor_copy(kT2[64:128, m, :], src)
        qk_items.append(restack)
        qts = (qkst, qT2)

        # even C-chunks -> PE col group 0-1 -> bank A; odd chunks -> col
        # group 2-3 -> bank B; the two chains run concurrently (separate
        # banks so each keeps its own psum accumulation group)
        vpsa = ps_va.tile([128, 512], F32, tag="va", name=f"vpsa{j}")
        vpsb = ps_vb.tile([128, 512], F32, tag="vb", name=f"vpsb{j}")
        for r in range(4):
            def vproj(r=r, vpsa=vpsa, vpsb=vpsb, j=j):
                nc.tensor.matmul(vpsa[0:64, :], lhsT=wv_c(2 * r),
                                 rhs=xs[j][:, 2 * r, :],
                                 start=(r == 0), stop=(r == 3),
                                 tile_position=(0, 0))
                nc.tensor.matmul(vpsb[64:128, :], lhsT=wv_c(2 * r + 1),
                                 rhs=xs[j][:, 2 * r + 1, :],
                                 start=(r == 0), stop=(r == 3),
                                 tile_position=(0, 64))
            vp_items.append(vproj)

        vth = vpool.tile([128, 512], BF16, tag="vth", name=f"vth{j}")

        def vcast(vpsa=vpsa, vpsb=vpsb, vth=vth):
            nc.vector.tensor_copy(vth[0:64, :], vpsa[0:64, :])
            nc.vector.tensor_copy(vth[64:128, :], vpsb[64:128, :])
        v_items.append(vcast)

        for t in range(4):
            def vnat(t=t, vth=vth, j=j):
                vpa = ps_va.tile([128, H], F32, tag="va", name=f"vpa{j}_{t}")
                vpb = ps_vb.tile([128, H], F32, tag="vb", name=f"vpb{j}_{t}")
                nc.tensor.matmul(vpa,
                                 lhsT=vth[0:64, t * 128:(t + 1) * 128],
                                 rhs=i64[0:64, :], start=True, stop=True)
                nc.tensor.matmul(vpb,
                                 lhsT=vth[64:128, t * 128:(t + 1) * 128],
                                 rhs=i64[64:128, :], start=True, stop=True)
                # DVE may read only ONE PSUM operand per instruction
                nc.vector.tensor_copy(v1[:, 4 * j + t, 0:H], vpa)
                nc.vector.tensor_add(v1[:, 4 * j + t, 0:H],
                                     v1[:, 4 * j + t, 0:H], vpb)
            v_items.append(vnat)
        return qk_items, vp_items, v_items, qts

    # ------------------------------------------------------------------
    def emit_s_pair(j, m, qts, pt_pool, done):
        """S^T pair m of slice j: 2 row-packed MMs + one exp + masks.
        Appends (pt, n0, m) to `done` for the later AV phase."""
        qkst, qT2 = qts
        n0 = (2 * m - 4 * j) * 128 if m >= 2 * j else 0
        sp = ps_s.tile([128, 2, 512], F32, tag="s", name=f"sp{j}_{m}")
        pt = pt_pool.tile([128, 2, 512], BF16, tag="pt", name=f"pt{j}_{m}")
        nc.tensor.matmul(sp[:, 0, n0:512], lhsT=kT2[0:64, m, :],
                         rhs=qkst[0:64, n0:512],
                         start=True, stop=True)
        nc.tensor.matmul(sp[:, 1, n0:512], lhsT=kT2[64:128, m, :],
                         rhs=qT2[64:128, n0:512],
                         start=True, stop=True)
        nc.scalar.activation(pt[:, :, n0:512], sp[:, :, n0:512],
                             mybir.ActivationFunctionType.Exp,
                             scale=SCALE)
        if m >= 2 * j:  # diagonal pair: tri-mask each diagonal block
            # (the exp'd strip [n0:n0+128) of the hi half is never read
            # by AV -- av starts at n0+128 there -- so no zeroing needed)
            nc.vector.tensor_mul(pt[:, 0, n0:n0 + 128],
                                 pt[:, 0, n0:n0 + 128], tri)
            nc.vector.tensor_mul(pt[:, 1, n0 + 128:n0 + 256],
                                 pt[:, 1, n0 + 128:n0 + 256], tri)
        done.append((pt, n0, m))

    def make_av_emitter(j):
        av = ps_av.tile([H + 1, 512], F32, tag="av", name=f"av{j}")

        def emit_av(pt, n0, m):
            i_lo, i_hi = 2 * m, 2 * m + 1
            nc.tensor.matmul(av[:, n0:512], lhsT=v1[:, i_lo, :],
                             rhs=pt[:, 0, n0:512],
                             start=(i_lo == 0), stop=False)
            n0h = n0 + 128 if m >= 2 * j else 0
            nc.tensor.matmul(av[:, n0h:512], lhsT=v1[:, i_hi, :],
                             rhs=pt[:, 1, n0h:512],
                             start=False, stop=(i_hi == 4 * j + 3))
        return av, emit_av

    def emit_epilogue(j, av):
        outsb = opool.tile([H + 1, 512], F32, tag="osb", name=f"osb{j}")
        nc.vector.tensor_copy(outsb, av)
        nc.sync.dma_start(out=out_d[:, j * 512:(j + 1) * 512], in_=outsb)

    def emit_attention(j, qts, pending):
        """S^T pairs + exp + masks + lag-2 AV, with `pending` (other work:
        next slice's projections, or the tail slice's S pairs) interleaved
        between pairs to keep the PE fed during exp waits."""
        av, emit_av = make_av_emitter(j)
        npair = 2 * j + 2
        done = []
        navd = 0
        for m in range(npair):
            emit_s_pair(j, m, qts, pts, done)
            if pending:
                take = max(1, -(-len(pending) // (npair - m)))
                for _ in range(min(take, len(pending))):
                    pending.pop(0)()
            if len(done) - navd > 2:
                emit_av(*done[navd])
                navd += 1
        for a in done[navd:]:
            emit_av(*a)
        for fn in pending:
            fn()
        emit_epilogue(j, av)

    # ------------------------------------------------------------------
    # Attention runs in natural slice order 0,1,2,3 -- this matches the
    # sequential arrival of the x-slice DMAs, so the exp stream on ScalarE
    # (the serial 21us backbone of the kernel) never starves. V work and
    # the next slice's q/k projection ride the attention interleave.
    qk0, vp0, v0, qts0 = emit_prologue(0)
    for fn in qk0:
        fn()
    qk1, vp1, v1i, qts1 = emit_prologue(1)
    emit_attention(0, qts0, vp0 + qk1 + v0)
    qk2, vp2, v2, qts2 = emit_prologue(2)
    emit_attention(1, qts1, vp1 + qk2 + v1i)
    qk3, vp3, v3, qts3 = emit_prologue(3)
    emit_attention(2, qts2, vp2 + qk3 + v2)
    emit_attention(3, qts3, vp3 + v3)


_CACHED = {}


def _get_nc():
    if "nc" not in _CACHED:
        from contextlib import ExitStack
        nc = bacc.Bacc("TRN2", target_bir_lowering=False, debug=False,
                       num_devices=B)
        with tile.TileContext(nc) as tc:
            with ExitStack() as ctx:
                build_attention(nc, tc, ctx)
        nc.compile()
        _CACHED["nc"] = nc
    return _CACHED["nc"]


def make_cpk(Wq, Wk, Wv):
    wqk = np.concatenate([np.asarray(Wq), np.asarray(Wk)],
                         axis=1).astype(np.float32)        # [C, 128]
    wqk_p = wqk.reshape(NCB, 128, 128).transpose(1, 0, 2).reshape(128, 1024)
    wv_p = np.asarray(Wv, dtype=np.float32).reshape(
        NCB, 128, H).transpose(1, 0, 2).reshape(128, NCB * H)
    i64_2 = np.concatenate([np.eye(64, dtype=np.float32)] * 2, axis=0)
    tri = np.triu(np.ones((128, 128), dtype=np.float32))
    cpk = np.concatenate([wqk_p, wv_p, i64_2, tri], axis=1)
    assert cpk.shape == (128, CPK_END)
    return np.ascontiguousarray(cpk).astype(npbf16)


def prep_in_maps(inputs, Wq, Wk, Wv):
    cpk = make_cpk(Wq, Wk, Wv)
    in_maps = []
    for b in range(B):
        xb = np.asarray(inputs[b], dtype=np.float32).astype(npbf16)
        # [T, C] -> [C, T] -> (c p) (j t) -> [p j c t]
        xh = np.ascontiguousarray(
            xb.T.reshape(NCB, 128, NJ, 512).transpose(1, 2, 0, 3))
        in_maps.append({"xT": xh, "cpk": cpk})
    return in_maps


def finish(res):
    outs = []
    for b in range(B):
        oT = np.asarray(res.results[b]["outT"], dtype=np.float32)
        outs.append((oT[:H] / oT[H:H + 1]).T)
    return np.stack(outs, axis=0).astype(np.float32)


def kernel(inputs, Wq, Wk, Wv):
    in_maps = prep_in_maps(np.asarray(inputs), Wq, Wk, Wv)
    nc = _get_nc()
    res = run_bass_kernel_spmd(nc, in_maps, core_ids=list(range(B)))
    return finish(res)
